# revision 2
# baseline (speedup 1.0000x reference)
"""Bilinear RGGB demosaic (Bayer -> RGB) on 8 Trainium2 NeuronCores, v6.

Image n -> core n. Host quantizes x to round(x*255) uint8 and
de-interleaves each image into [1024 row-pairs, 4104] with segments
[Ee |Z4| Eo | Oe |Z4| Oo] (E/O = even/odd row, e/o = even/odd col; Z4 =
4 zero pad cols serving the horizontal-shift edge reads, so there are no
edge-fix ops). Loads run on the SWDGE (gpsimd) ring with dtype cast
u8 -> fp16, so input arrives 255-scaled at half the bytes with zero
engine cost; all on-chip fp16 values are multiples of 0.25 up to 510 --
exact -- so the only error is the two 8-bit quantizations (~4e-3 rel vs
the 2e-2 gate).

Per band: 4 contiguous DVE adds form the horizontal pair sums; 12 fp16
matmuls (6 streams x 2 chunks of 512, banded 128x128 stationaries)
produce the vertical/diagonal phases in f32 PSUM; the R and B streams
pair up in shared [128,1024] PSUM tiles ([Roe|Roo], [Bee|Beo]) matching
a chunk-interleaved DRAM layout so each pair drains with a single
1024-wide ACT copy (f32 -> u8). Output: one u8 RGB tile [128, 12288]
per band, stored in 3 x 1 MB-class DMAs on the qSP HWDGE ring (loads
and stores on separate rings). Host re-interleaves + dequantizes /255.
"""

import sys

sys.path.insert(0, "/opt/trn_rl_repo")

import numpy as np

import concourse.bass as bass
import concourse.tile as tile
from concourse import mybir
from concourse.alu_op_type import AluOpType

F16 = mybir.dt.float16
F32 = mybir.dt.float32
U8 = mybir.dt.uint8
N_CORES = 8
H = 2048
W = 2048
NP = H // 2  # 1024 row pairs
HW = W // 2  # 1024 cols per parity
PAD = 4
IW = 4 * HW + 2 * PAD  # in-row width incl. two pad blocks
EE0 = 0
EO0 = HW + PAD
OE0 = 2 * HW + PAD
OO0 = 3 * HW + 2 * PAD


def split_sync_waits(nc, max_waits=1):
    """This walrus build rejects instructions carrying more than
    `max_waits` sync-wait commands. Hoist excess waits onto same-engine
    NoOps inserted immediately before the over-subscribed instruction
    (waiting earlier on the same queue is semantically conservative)."""
    for fn in nc.m.functions:
        for bb in fn.blocks:
            insts = bb.instructions
            i = 0
            while i < len(insts):
                inst = insts[i]
                si = inst.sync_info
                waits = list(si.on_wait) if si and si.on_wait else []
                if len(waits) > max_waits:
                    si.on_wait = waits[:max_waits]
                    excess = waits[max_waits:]
                    for j in range(0, len(excess), max_waits):
                        nop = mybir.InstNoOp(
                            name=nc.get_next_instruction_name(), ins=[], outs=[]
                        )
                        nop.engine = inst.engine
                        nop.sync_info = mybir.SyncInfo(
                            on_wait=excess[j : j + max_waits], on_update=[]
                        )
                        nc.register_instruction(nop)
                        insts.insert(i, nop)
                        i += 1
                i += 1


def const_arrays():
    # matmul computes lhsT.T @ rhs: out[m] = sum_k lhsT[k, m] * in[k], so
    # the "down" sum in[m]+in[m+1] needs the k=-1 subdiagonal and vice versa.
    d05 = 0.5 * (np.eye(128) + np.eye(128, k=-1))
    d025 = 0.25 * (np.eye(128) + np.eye(128, k=-1))
    u05 = 0.5 * (np.eye(128) + np.eye(128, k=1))
    u025 = 0.25 * (np.eye(128) + np.eye(128, k=1))
    return np.concatenate([d05, d025, u05, u025], axis=1).astype(np.float16)


def band_plan(npairs):
    """Bands of 128 row-pairs advancing ~126 pairs: each band stores only
    the pair range whose vertical neighbors are in-tile, so no halo or
    seam-fix work is needed. Returns [(start_pair, store_lo, store_hi)]."""
    plan = []
    covered = 0
    while covered < npairs:
        q = 0 if covered == 0 else min(covered - 1, npairs - 128)
        lo = covered - q
        hi = 128 if q + 128 >= npairs else 127
        plan.append((q, lo, hi))
        covered = q + hi
    return plan


def build_program(store_pieces=3, in_bufs=3, out_bufs=2, vdrain=False, alt_stores=False, load_mode="swdge"):
    nc = bass.Bass("TRN2", target_bir_lowering=False, debug=False)
    x = nc.dram_tensor(
        "x", [NP, IW], U8 if load_mode != "fp16" else F16, kind="ExternalInput"
    ).ap()
    cmm = nc.dram_tensor("cmm", [128, 512], F16, kind="ExternalInput").ap()
    out = nc.dram_tensor("out", [NP, 12 * HW], U8, kind="ExternalOutput").ap()

    plan = band_plan(NP)
    h = HW
    cw = 512

    st = nc.sync  # stores: qSP HWDGE ring
    V = nc.vector
    S = nc.scalar
    mu, ad = AluOpType.mult, AluOpType.add

    with tile.TileContext(nc) as tc:
        with (
            tc.tile_pool(name="consts", bufs=1) as cpool,
            tc.tile_pool(name="inp", bufs=in_bufs) as ipool,
            tc.tile_pool(name="hsum", bufs=2) as hpool,
            tc.tile_pool(name="psum", bufs=1, space="PSUM") as ppool,
            tc.tile_pool(name="outs", bufs=out_bufs) as opool,
        ):
            cM = cpool.tile([128, 512], F16)
            nc.scalar.dma_start(cM[:], cmm[:])
            D05 = cM[:, 0:128]
            D025 = cM[:, 128:256]
            U05 = cM[:, 256:384]
            U025 = cM[:, 384:512]

            for q, lo, hi in plan:
                IN = ipool.tile([128, IW], F16, tag="in")
                if load_mode == "fp16":
                    nc.scalar.dma_start(IN[:], x[q : q + 128, :])
                else:
                    nc.gpsimd.dma_start(IN[:], x[q : q + 128, :])  # SWDGE u8->fp16

                Ee = IN[:, EE0 : EE0 + h]
                Eo = IN[:, EO0 : EO0 + h]
                Oe = IN[:, OE0 : OE0 + h]
                Oo = IN[:, OO0 : OO0 + h]

                # Horizontal pair sums; pad columns make every edge exact.
                He = hpool.tile([128, h], F16, tag="he")  # Ee[i]+Ee[i+1]
                Ho = hpool.tile([128, h], F16, tag="ho")  # Oo[i-1]+Oo[i]
                Hu = hpool.tile([128, h], F16, tag="hu")  # Eo[i-1]+Eo[i]
                Hd = hpool.tile([128, h], F16, tag="hd")  # Oe[i]+Oe[i+1]
                V.tensor_add(He[:], Ee[:, :], IN[:, EE0 + 1 : EE0 + 1 + h])
                V.tensor_add(Ho[:], IN[:, OO0 - 1 : OO0 - 1 + h], Oo[:, :])
                V.tensor_add(Hu[:], IN[:, EO0 - 1 : EO0 - 1 + h], Eo[:, :])
                V.tensor_add(Hd[:], Oe[:, :], IN[:, OE0 + 1 : OE0 + 1 + h])

                RGB = opool.tile([128, 12 * h], U8, tag="rgb")
                R0, G0, B0 = 0, 4 * h, 8 * h

                # Non-PSUM phases.
                V.tensor_copy(RGB[:, R0 : R0 + h], Ee[:])  # R-ee
                V.tensor_copy(RGB[:, B0 + 3 * h : B0 + 4 * h], Oo[:])  # B-oo
                # G-eo and G-oe: Eo,Oe adjacent in IN and in the G block
                S.copy(RGB[:, G0 + h : G0 + 3 * h], IN[:, EO0 : EO0 + 2 * h])
                V.tensor_scalar_mul(RGB[:, R0 + h : R0 + 2 * h], He[:], 0.5)  # R-eo
                V.tensor_scalar_mul(RGB[:, B0 + 2 * h : B0 + 3 * h], Ho[:], 0.5)  # B-oe

                # PSUM streams: [Roe|Roo] and [Bee|Beo] pair per chunk in
                # shared 2-bank tiles matching the chunk-interleaved DRAM
                # layout; G streams feed stt consumers directly.
                for c in (0, cw):
                    s = slice(c, c + cw)
                    Prr = ppool.tile([128, 2 * cw], F32, tag="rr")
                    Pbb = ppool.tile([128, 2 * cw], F32, tag="bb")
                    Pge = ppool.tile([128, cw], F32, tag="ge")
                    Pgo = ppool.tile([128, cw], F32, tag="go")
                    nc.tensor.matmul(Prr[:, 0:cw], D05[:], Ee[:, s], start=True, stop=True)
                    nc.tensor.matmul(Prr[:, cw : 2 * cw], D025[:], He[:, s], start=True, stop=True)
                    nc.tensor.matmul(Pgo[:], D025[:], Eo[:, s], start=True, stop=True)
                    nc.tensor.matmul(Pge[:], U025[:], Oe[:, s], start=True, stop=True)
                    nc.tensor.matmul(Pbb[:, 0:cw], U025[:], Ho[:, s], start=True, stop=True)
                    nc.tensor.matmul(Pbb[:, cw : 2 * cw], U05[:], Oo[:, s], start=True, stop=True)

                    # single-op drains into chunk-interleaved R/B blocks
                    if vdrain:
                        V.tensor_copy(RGB[:, R0 + 2 * h + 2 * c : R0 + 2 * h + 2 * c + 2 * cw], Prr[:])
                        V.tensor_copy(RGB[:, B0 + 2 * c : B0 + 2 * c + 2 * cw], Pbb[:])
                    else:
                        S.copy(RGB[:, R0 + 2 * h + 2 * c : R0 + 2 * h + 2 * c + 2 * cw], Prr[:])
                        S.copy(RGB[:, B0 + 2 * c : B0 + 2 * c + 2 * cw], Pbb[:])
                    # G-ee / G-oo
                    V.scalar_tensor_tensor(
                        RGB[:, G0 + c : G0 + c + cw], Hu[:, s], 0.25, Pge[:], mu, ad
                    )
                    V.scalar_tensor_tensor(
                        RGB[:, G0 + 3 * h + c : G0 + 3 * h + c + cw],
                        Hd[:, s], 0.25, Pgo[:], mu, ad,
                    )

                step = 12 * h // store_pieces
                for j in range(store_pieces):
                    eng = S if (alt_stores and j % 2 == 1) else st
                    eng.dma_start(
                        out[q + lo : q + hi, j * step : (j + 1) * step],
                        RGB[lo:hi, j * step : (j + 1) * step],
                    )

    split_sync_waits(nc)
    return nc


_CACHE = {}

TRACE = False
LAST_RESULT = None


def _get_program():
    import os

    key = (
        int(os.environ.get("K6SPLITN", "3")),
        int(os.environ.get("K6INBUFS", "3")),
        int(os.environ.get("K6OUTBUFS", "2")),
        os.environ.get("K6VDRAIN", "0") == "1",
        os.environ.get("K6STALT", "0") == "1",
        os.environ.get("K7LOAD", "swdge"),
    )
    if key not in _CACHE:
        _CACHE[key] = build_program(*key)
    return _CACHE[key]


def kernel(x: np.ndarray) -> np.ndarray:
    global LAST_RESULT
    from concourse.bass_utils import run_bass_kernel_spmd

    n, _, hh, ww = x.shape
    assert (n, hh, ww) == (N_CORES, H, W), x.shape
    nc = _get_program()
    cmm = const_arrays()
    import os
    fp16_in = os.environ.get("K7LOAD", "swdge") == "fp16"
    in_maps = []
    for i in range(N_CORES):
        img = np.rint(x[i, 0] * 255.0).astype(np.uint8)
        # de-interleave: [1024, 2, 1024, 2] -> [p, r, s, i] -> [Ee|Eo|Oe|Oo]
        seg = np.ascontiguousarray(
            img.reshape(NP, 2, HW, 2).transpose(0, 1, 3, 2)
        ).reshape(NP, 4 * HW)
        d = np.zeros((NP, IW), np.float16 if fp16_in else np.uint8)
        d[:, EE0 : EE0 + HW] = seg[:, 0:HW]
        d[:, EO0 : EO0 + HW] = seg[:, HW : 2 * HW]
        d[:, OE0 : OE0 + HW] = seg[:, 2 * HW : 3 * HW]
        d[:, OO0 : OO0 + HW] = seg[:, 3 * HW : 4 * HW]
        in_maps.append({"x": d, "cmm": cmm})
    res = run_bass_kernel_spmd(
        nc, in_maps, core_ids=list(range(N_CORES)), trace=TRACE
    )
    LAST_RESULT = res
    outs = []
    inv = 1.0 / 255.0
    for i in range(N_CORES):
        o = res.results[i]["out"].reshape(NP, 3, 4 * HW)
        # undo chunk interleaving of R (oe/oo) and B (ee/eo) blocks
        ph = np.empty((3, NP, 4, HW), np.uint8)  # [c, p, phase(ee,eo,oe,oo), i]
        R, G, B = o[:, 0], o[:, 1], o[:, 2]
        ph[0, :, 0] = R[:, 0:HW]
        ph[0, :, 1] = R[:, HW : 2 * HW]
        ph[0, :, 2, 0:512] = R[:, 2 * HW : 2 * HW + 512]
        ph[0, :, 3, 0:512] = R[:, 2 * HW + 512 : 3 * HW]
        ph[0, :, 2, 512:] = R[:, 3 * HW : 3 * HW + 512]
        ph[0, :, 3, 512:] = R[:, 3 * HW + 512 : 4 * HW]
        ph[1] = G.reshape(NP, 4, HW)
        ph[2, :, 0, 0:512] = B[:, 0:512]
        ph[2, :, 1, 0:512] = B[:, 512:HW]
        ph[2, :, 0, 512:] = B[:, HW : HW + 512]
        ph[2, :, 1, 512:] = B[:, HW + 512 : 2 * HW]
        ph[2, :, 2] = B[:, 2 * HW : 3 * HW]
        ph[2, :, 3] = B[:, 3 * HW : 4 * HW]
        phr = ph.reshape(3, NP, 2, 2, HW)  # [c, p, r, s, i]
        full = np.ascontiguousarray(
            (phr.astype(np.float32) * inv).transpose(0, 1, 2, 4, 3)
        ).reshape(3, H, W)
        outs.append(full[None])
    return np.concatenate(outs, axis=0)


# revision 3
# speedup vs baseline: 1.0584x; 1.0584x over previous
"""Bilinear RGGB demosaic (Bayer -> RGB) on 8 Trainium2 NeuronCores, v6.

Image n -> core n. Host quantizes x to round(x*255) uint8 and
de-interleaves each image into [1024 row-pairs, 4104] with segments
[Ee |Z4| Eo | Oe |Z4| Oo] (E/O = even/odd row, e/o = even/odd col; Z4 =
4 zero pad cols serving the horizontal-shift edge reads, so there are no
edge-fix ops). Loads run on the SWDGE (gpsimd) ring with dtype cast
u8 -> fp16, so input arrives 255-scaled at half the bytes with zero
engine cost; all on-chip fp16 values are multiples of 0.25 up to 510 --
exact -- so the only error is the two 8-bit quantizations (~4e-3 rel vs
the 2e-2 gate).

Per band: 4 contiguous DVE adds form the horizontal pair sums; 12 fp16
matmuls (6 streams x 2 chunks of 512, banded 128x128 stationaries)
produce the vertical/diagonal phases in f32 PSUM; the R and B streams
pair up in shared [128,1024] PSUM tiles ([Roe|Roo], [Bee|Beo]) matching
a chunk-interleaved DRAM layout so each pair drains with a single
1024-wide ACT copy (f32 -> u8). Output: one u8 RGB tile [128, 12288]
per band, stored in 3 x 1 MB-class DMAs on the qSP HWDGE ring (loads
and stores on separate rings). Host re-interleaves + dequantizes /255.
"""

import sys

sys.path.insert(0, "/opt/trn_rl_repo")

import numpy as np

import concourse.bass as bass
import concourse.tile as tile
from concourse import mybir
from concourse.alu_op_type import AluOpType

F16 = mybir.dt.float16
F32 = mybir.dt.float32
U8 = mybir.dt.uint8
N_CORES = 8
H = 2048
W = 2048
NP = H // 2  # 1024 row pairs
HW = W // 2  # 1024 cols per parity
PAD = 4
IW = 4 * HW + 2 * PAD  # in-row width incl. two pad blocks
EE0 = 0
EO0 = HW + PAD
OE0 = 2 * HW + PAD
OO0 = 3 * HW + 2 * PAD


def split_sync_waits(nc, max_waits=1):
    """This walrus build rejects instructions carrying more than
    `max_waits` sync-wait commands. Hoist excess waits onto same-engine
    NoOps inserted immediately before the over-subscribed instruction
    (waiting earlier on the same queue is semantically conservative)."""
    for fn in nc.m.functions:
        for bb in fn.blocks:
            insts = bb.instructions
            i = 0
            while i < len(insts):
                inst = insts[i]
                si = inst.sync_info
                waits = list(si.on_wait) if si and si.on_wait else []
                if len(waits) > max_waits:
                    si.on_wait = waits[:max_waits]
                    excess = waits[max_waits:]
                    for j in range(0, len(excess), max_waits):
                        nop = mybir.InstNoOp(
                            name=nc.get_next_instruction_name(), ins=[], outs=[]
                        )
                        nop.engine = inst.engine
                        nop.sync_info = mybir.SyncInfo(
                            on_wait=excess[j : j + max_waits], on_update=[]
                        )
                        nc.register_instruction(nop)
                        insts.insert(i, nop)
                        i += 1
                i += 1


def const_arrays():
    # matmul computes lhsT.T @ rhs: out[m] = sum_k lhsT[k, m] * in[k], so
    # the "down" sum in[m]+in[m+1] needs the k=-1 subdiagonal and vice versa.
    d05 = 0.5 * (np.eye(128) + np.eye(128, k=-1))
    d025 = 0.25 * (np.eye(128) + np.eye(128, k=-1))
    u05 = 0.5 * (np.eye(128) + np.eye(128, k=1))
    u025 = 0.25 * (np.eye(128) + np.eye(128, k=1))
    return np.concatenate([d05, d025, u05, u025], axis=1).astype(np.float16)


def band_plan(npairs):
    """Bands of 128 row-pairs advancing ~126 pairs: each band stores only
    the pair range whose vertical neighbors are in-tile, so no halo or
    seam-fix work is needed. Returns [(start_pair, store_lo, store_hi)]."""
    plan = []
    covered = 0
    while covered < npairs:
        q = 0 if covered == 0 else min(covered - 1, npairs - 128)
        lo = covered - q
        hi = 128 if q + 128 >= npairs else 127
        plan.append((q, lo, hi))
        covered = q + hi
    return plan


def build_program(store_pieces=3, in_bufs=3, out_bufs=2, vdrain=False, alt_stores=False, load_mode="swdge"):
    nc = bass.Bass("TRN2", target_bir_lowering=False, debug=False)
    x = nc.dram_tensor(
        "x", [NP, IW], U8 if load_mode != "fp16" else F16, kind="ExternalInput"
    ).ap()
    cmm = nc.dram_tensor("cmm", [128, 512], F16, kind="ExternalInput").ap()
    out = nc.dram_tensor("out", [NP, 12 * HW], U8, kind="ExternalOutput").ap()

    plan = band_plan(NP)
    h = HW
    cw = 512

    import os
    st = getattr(nc, os.environ.get('K7STORE', 'gpsimd'))  # stores ring
    V = nc.vector
    S = nc.scalar
    mu, ad = AluOpType.mult, AluOpType.add

    with tile.TileContext(nc) as tc:
        with (
            tc.tile_pool(name="consts", bufs=1) as cpool,
            tc.tile_pool(name="inp", bufs=in_bufs) as ipool,
            tc.tile_pool(name="hsum", bufs=2) as hpool,
            tc.tile_pool(name="psum", bufs=1, space="PSUM") as ppool,
            tc.tile_pool(name="outs", bufs=out_bufs) as opool,
        ):
            cM = cpool.tile([128, 512], F16)
            nc.scalar.dma_start(cM[:], cmm[:])
            D05 = cM[:, 0:128]
            D025 = cM[:, 128:256]
            U05 = cM[:, 256:384]
            U025 = cM[:, 384:512]

            for q, lo, hi in plan:
                IN = ipool.tile([128, IW], F16, tag="in")
                if load_mode == "fp16":
                    nc.scalar.dma_start(IN[:], x[q : q + 128, :])
                else:
                    nc.gpsimd.dma_start(IN[:], x[q : q + 128, :])  # SWDGE u8->fp16

                Ee = IN[:, EE0 : EE0 + h]
                Eo = IN[:, EO0 : EO0 + h]
                Oe = IN[:, OE0 : OE0 + h]
                Oo = IN[:, OO0 : OO0 + h]

                # Horizontal pair sums; pad columns make every edge exact.
                He = hpool.tile([128, h], F16, tag="he")  # Ee[i]+Ee[i+1]
                Ho = hpool.tile([128, h], F16, tag="ho")  # Oo[i-1]+Oo[i]
                Hu = hpool.tile([128, h], F16, tag="hu")  # Eo[i-1]+Eo[i]
                Hd = hpool.tile([128, h], F16, tag="hd")  # Oe[i]+Oe[i+1]
                V.tensor_add(He[:], Ee[:, :], IN[:, EE0 + 1 : EE0 + 1 + h])
                V.tensor_add(Ho[:], IN[:, OO0 - 1 : OO0 - 1 + h], Oo[:, :])
                V.tensor_add(Hu[:], IN[:, EO0 - 1 : EO0 - 1 + h], Eo[:, :])
                V.tensor_add(Hd[:], Oe[:, :], IN[:, OE0 + 1 : OE0 + 1 + h])

                RGB = opool.tile([128, 12 * h], U8, tag="rgb")
                R0, G0, B0 = 0, 4 * h, 8 * h

                # Non-PSUM phases.
                V.tensor_copy(RGB[:, R0 : R0 + h], Ee[:])  # R-ee
                V.tensor_copy(RGB[:, B0 + 3 * h : B0 + 4 * h], Oo[:])  # B-oo
                # G-eo and G-oe: Eo,Oe adjacent in IN and in the G block
                S.copy(RGB[:, G0 + h : G0 + 3 * h], IN[:, EO0 : EO0 + 2 * h])
                V.tensor_scalar_mul(RGB[:, R0 + h : R0 + 2 * h], He[:], 0.5)  # R-eo
                V.tensor_scalar_mul(RGB[:, B0 + 2 * h : B0 + 3 * h], Ho[:], 0.5)  # B-oe

                # PSUM streams: [Roe|Roo] and [Bee|Beo] pair per chunk in
                # shared 2-bank tiles matching the chunk-interleaved DRAM
                # layout; G streams feed stt consumers directly.
                for c in (0, cw):
                    s = slice(c, c + cw)
                    Prr = ppool.tile([128, 2 * cw], F32, tag="rr")
                    Pbb = ppool.tile([128, 2 * cw], F32, tag="bb")
                    Pge = ppool.tile([128, cw], F32, tag="ge")
                    Pgo = ppool.tile([128, cw], F32, tag="go")
                    nc.tensor.matmul(Prr[:, 0:cw], D05[:], Ee[:, s], start=True, stop=True)
                    nc.tensor.matmul(Prr[:, cw : 2 * cw], D025[:], He[:, s], start=True, stop=True)
                    nc.tensor.matmul(Pgo[:], D025[:], Eo[:, s], start=True, stop=True)
                    nc.tensor.matmul(Pge[:], U025[:], Oe[:, s], start=True, stop=True)
                    nc.tensor.matmul(Pbb[:, 0:cw], U025[:], Ho[:, s], start=True, stop=True)
                    nc.tensor.matmul(Pbb[:, cw : 2 * cw], U05[:], Oo[:, s], start=True, stop=True)

                    # single-op drains into chunk-interleaved R/B blocks
                    if vdrain:
                        V.tensor_copy(RGB[:, R0 + 2 * h + 2 * c : R0 + 2 * h + 2 * c + 2 * cw], Prr[:])
                        V.tensor_copy(RGB[:, B0 + 2 * c : B0 + 2 * c + 2 * cw], Pbb[:])
                    else:
                        S.copy(RGB[:, R0 + 2 * h + 2 * c : R0 + 2 * h + 2 * c + 2 * cw], Prr[:])
                        S.copy(RGB[:, B0 + 2 * c : B0 + 2 * c + 2 * cw], Pbb[:])
                    # G-ee / G-oo
                    V.scalar_tensor_tensor(
                        RGB[:, G0 + c : G0 + c + cw], Hu[:, s], 0.25, Pge[:], mu, ad
                    )
                    V.scalar_tensor_tensor(
                        RGB[:, G0 + 3 * h + c : G0 + 3 * h + c + cw],
                        Hd[:, s], 0.25, Pgo[:], mu, ad,
                    )

                step = 12 * h // store_pieces
                for j in range(store_pieces):
                    eng = S if (alt_stores and j % 2 == 1) else st
                    eng.dma_start(
                        out[q + lo : q + hi, j * step : (j + 1) * step],
                        RGB[lo:hi, j * step : (j + 1) * step],
                    )

    split_sync_waits(nc)
    return nc


_CACHE = {}

TRACE = False
LAST_RESULT = None


def _get_program():
    import os

    key = (
        int(os.environ.get("K6SPLITN", "3")),
        int(os.environ.get("K6INBUFS", "3")),
        int(os.environ.get("K6OUTBUFS", "2")),
        os.environ.get("K6VDRAIN", "0") == "1",
        os.environ.get("K6STALT", "0") == "1",
        os.environ.get("K7LOAD", "swdge"),
    )
    if key not in _CACHE:
        _CACHE[key] = build_program(*key)
    return _CACHE[key]


def kernel(x: np.ndarray) -> np.ndarray:
    global LAST_RESULT
    from concourse.bass_utils import run_bass_kernel_spmd

    n, _, hh, ww = x.shape
    assert (n, hh, ww) == (N_CORES, H, W), x.shape
    nc = _get_program()
    cmm = const_arrays()
    import os
    fp16_in = os.environ.get("K7LOAD", "swdge") == "fp16"
    in_maps = []
    for i in range(N_CORES):
        img = np.rint(x[i, 0] * 255.0).astype(np.uint8)
        # de-interleave: [1024, 2, 1024, 2] -> [p, r, s, i] -> [Ee|Eo|Oe|Oo]
        seg = np.ascontiguousarray(
            img.reshape(NP, 2, HW, 2).transpose(0, 1, 3, 2)
        ).reshape(NP, 4 * HW)
        d = np.zeros((NP, IW), np.float16 if fp16_in else np.uint8)
        d[:, EE0 : EE0 + HW] = seg[:, 0:HW]
        d[:, EO0 : EO0 + HW] = seg[:, HW : 2 * HW]
        d[:, OE0 : OE0 + HW] = seg[:, 2 * HW : 3 * HW]
        d[:, OO0 : OO0 + HW] = seg[:, 3 * HW : 4 * HW]
        in_maps.append({"x": d, "cmm": cmm})
    res = run_bass_kernel_spmd(
        nc, in_maps, core_ids=list(range(N_CORES)), trace=TRACE
    )
    LAST_RESULT = res
    outs = []
    inv = 1.0 / 255.0
    for i in range(N_CORES):
        o = res.results[i]["out"].reshape(NP, 3, 4 * HW)
        # undo chunk interleaving of R (oe/oo) and B (ee/eo) blocks
        ph = np.empty((3, NP, 4, HW), np.uint8)  # [c, p, phase(ee,eo,oe,oo), i]
        R, G, B = o[:, 0], o[:, 1], o[:, 2]
        ph[0, :, 0] = R[:, 0:HW]
        ph[0, :, 1] = R[:, HW : 2 * HW]
        ph[0, :, 2, 0:512] = R[:, 2 * HW : 2 * HW + 512]
        ph[0, :, 3, 0:512] = R[:, 2 * HW + 512 : 3 * HW]
        ph[0, :, 2, 512:] = R[:, 3 * HW : 3 * HW + 512]
        ph[0, :, 3, 512:] = R[:, 3 * HW + 512 : 4 * HW]
        ph[1] = G.reshape(NP, 4, HW)
        ph[2, :, 0, 0:512] = B[:, 0:512]
        ph[2, :, 1, 0:512] = B[:, 512:HW]
        ph[2, :, 0, 512:] = B[:, HW : HW + 512]
        ph[2, :, 1, 512:] = B[:, HW + 512 : 2 * HW]
        ph[2, :, 2] = B[:, 2 * HW : 3 * HW]
        ph[2, :, 3] = B[:, 3 * HW : 4 * HW]
        phr = ph.reshape(3, NP, 2, 2, HW)  # [c, p, r, s, i]
        full = np.ascontiguousarray(
            (phr.astype(np.float32) * inv).transpose(0, 1, 2, 4, 3)
        ).reshape(3, H, W)
        outs.append(full[None])
    return np.concatenate(outs, axis=0)


# revision 4
# speedup vs baseline: 1.1409x; 1.0779x over previous
"""Bilinear RGGB demosaic (Bayer -> RGB) on 8 Trainium2 NeuronCores, v6.

Image n -> core n. Host quantizes x to round(x*255) uint8 and
de-interleaves each image into [1024 row-pairs, 4104] with segments
[Ee |Z4| Eo | Oe |Z4| Oo] (E/O = even/odd row, e/o = even/odd col; Z4 =
4 zero pad cols serving the horizontal-shift edge reads, so there are no
edge-fix ops). Loads run on the SWDGE (gpsimd) ring with dtype cast
u8 -> fp16, so input arrives 255-scaled at half the bytes with zero
engine cost; all on-chip fp16 values are multiples of 0.25 up to 510 --
exact -- so the only error is the two 8-bit quantizations (~4e-3 rel vs
the 2e-2 gate).

Per band: 4 contiguous DVE adds form the horizontal pair sums; 12 fp16
matmuls (6 streams x 2 chunks of 512, banded 128x128 stationaries)
produce the vertical/diagonal phases in f32 PSUM; the R and B streams
pair up in shared [128,1024] PSUM tiles ([Roe|Roo], [Bee|Beo]) matching
a chunk-interleaved DRAM layout so each pair drains with a single
1024-wide ACT copy (f32 -> u8). Output: one u8 RGB tile [128, 12288]
per band, stored in 3 x 1 MB-class DMAs on the qSP HWDGE ring (loads
and stores on separate rings). Host re-interleaves + dequantizes /255.
"""

import sys

sys.path.insert(0, "/opt/trn_rl_repo")

import numpy as np

import concourse.bass as bass
import concourse.tile as tile
from concourse import mybir
from concourse.alu_op_type import AluOpType

F16 = mybir.dt.float16
F32 = mybir.dt.float32
U8 = mybir.dt.uint8
N_CORES = 8
H = 2048
W = 2048
NP = H // 2  # 1024 row pairs
HW = W // 2  # 1024 cols per parity
PAD = 4
IW = 4 * HW + 2 * PAD  # in-row width incl. two pad blocks
EE0 = 0
EO0 = HW + PAD
OE0 = 2 * HW + PAD
OO0 = 3 * HW + 2 * PAD


def split_sync_waits(nc, max_waits=1):
    """This walrus build rejects instructions carrying more than
    `max_waits` sync-wait commands. Hoist excess waits onto same-engine
    NoOps inserted immediately before the over-subscribed instruction
    (waiting earlier on the same queue is semantically conservative)."""
    for fn in nc.m.functions:
        for bb in fn.blocks:
            insts = bb.instructions
            i = 0
            while i < len(insts):
                inst = insts[i]
                si = inst.sync_info
                waits = list(si.on_wait) if si and si.on_wait else []
                if len(waits) > max_waits:
                    si.on_wait = waits[:max_waits]
                    excess = waits[max_waits:]
                    for j in range(0, len(excess), max_waits):
                        nop = mybir.InstNoOp(
                            name=nc.get_next_instruction_name(), ins=[], outs=[]
                        )
                        nop.engine = inst.engine
                        nop.sync_info = mybir.SyncInfo(
                            on_wait=excess[j : j + max_waits], on_update=[]
                        )
                        nc.register_instruction(nop)
                        insts.insert(i, nop)
                        i += 1
                i += 1


def const_arrays():
    # matmul computes lhsT.T @ rhs: out[m] = sum_k lhsT[k, m] * in[k], so
    # the "down" sum in[m]+in[m+1] needs the k=-1 subdiagonal and vice versa.
    d05 = 0.5 * (np.eye(128) + np.eye(128, k=-1))
    d025 = 0.25 * (np.eye(128) + np.eye(128, k=-1))
    u05 = 0.5 * (np.eye(128) + np.eye(128, k=1))
    u025 = 0.25 * (np.eye(128) + np.eye(128, k=1))
    return np.concatenate([d05, d025, u05, u025], axis=1).astype(np.float16)


def band_plan(npairs):
    """Bands of 128 row-pairs advancing ~126 pairs: each band stores only
    the pair range whose vertical neighbors are in-tile, so no halo or
    seam-fix work is needed. Returns [(start_pair, store_lo, store_hi)]."""
    plan = []
    covered = 0
    while covered < npairs:
        q = 0 if covered == 0 else min(covered - 1, npairs - 128)
        lo = covered - q
        hi = 128 if q + 128 >= npairs else 127
        plan.append((q, lo, hi))
        covered = q + hi
    return plan


def build_program(store_pieces=3, in_bufs=3, out_bufs=2, vdrain=False, alt_stores=False, load_mode="swdge"):
    nc = bass.Bass("TRN2", target_bir_lowering=False, debug=False)
    x = nc.dram_tensor(
        "x", [NP, IW], U8 if load_mode != "fp16" else F16, kind="ExternalInput"
    ).ap()
    cmm = nc.dram_tensor("cmm", [128, 512], F16, kind="ExternalInput").ap()
    out = nc.dram_tensor("out", [NP, 12 * HW], U8, kind="ExternalOutput").ap()

    plan = band_plan(NP)
    h = HW
    cw = 512

    import os
    st = getattr(nc, os.environ.get('K7STORE', 'gpsimd'))  # stores ring
    V = nc.vector
    S = nc.scalar
    mu, ad = AluOpType.mult, AluOpType.add

    with tile.TileContext(nc) as tc:
        with (
            tc.tile_pool(name="consts", bufs=1) as cpool,
            tc.tile_pool(name="inp", bufs=in_bufs) as ipool,
            tc.tile_pool(name="hsum", bufs=2) as hpool,
            tc.tile_pool(name="psum", bufs=1, space="PSUM") as ppool,
            tc.tile_pool(name="outs", bufs=out_bufs) as opool,
        ):
            cM = cpool.tile([128, 512], F16)
            nc.scalar.dma_start(cM[:], cmm[:])
            D05 = cM[:, 0:128]
            D025 = cM[:, 128:256]
            U05 = cM[:, 256:384]
            U025 = cM[:, 384:512]

            for q, lo, hi in plan:
                IN = ipool.tile([128, IW], F16, tag="in")
                if load_mode == "fp16":
                    nc.scalar.dma_start(IN[:], x[q : q + 128, :])
                else:
                    nc.gpsimd.dma_start(IN[:], x[q : q + 128, :])  # SWDGE u8->fp16

                Ee = IN[:, EE0 : EE0 + h]
                Eo = IN[:, EO0 : EO0 + h]
                Oe = IN[:, OE0 : OE0 + h]
                Oo = IN[:, OO0 : OO0 + h]

                # Horizontal pair sums; pad columns make every edge exact.
                He = hpool.tile([128, h], F16, tag="he")  # Ee[i]+Ee[i+1]
                Ho = hpool.tile([128, h], F16, tag="ho")  # Oo[i-1]+Oo[i]
                Hu = hpool.tile([128, h], F16, tag="hu")  # Eo[i-1]+Eo[i]
                Hd = hpool.tile([128, h], F16, tag="hd")  # Oe[i]+Oe[i+1]
                V.tensor_add(He[:], Ee[:, :], IN[:, EE0 + 1 : EE0 + 1 + h])
                V.tensor_add(Ho[:], IN[:, OO0 - 1 : OO0 - 1 + h], Oo[:, :])
                V.tensor_add(Hu[:], IN[:, EO0 - 1 : EO0 - 1 + h], Eo[:, :])
                V.tensor_add(Hd[:], Oe[:, :], IN[:, OE0 + 1 : OE0 + 1 + h])

                RGB = opool.tile([128, 12 * h], U8, tag="rgb")
                R0, G0, B0 = 0, 4 * h, 8 * h

                # Non-PSUM phases.
                V.tensor_copy(RGB[:, R0 : R0 + h], Ee[:])  # R-ee
                V.tensor_copy(RGB[:, B0 + 3 * h : B0 + 4 * h], Oo[:])  # B-oo
                # G-eo and G-oe: Eo,Oe adjacent in IN and in the G block
                S.copy(RGB[:, G0 + h : G0 + 3 * h], IN[:, EO0 : EO0 + 2 * h])
                V.tensor_scalar_mul(RGB[:, R0 + h : R0 + 2 * h], He[:], 0.5)  # R-eo
                V.tensor_scalar_mul(RGB[:, B0 + 2 * h : B0 + 3 * h], Ho[:], 0.5)  # B-oe

                # PSUM streams: [Roe|Roo] and [Bee|Beo] pair per chunk in
                # shared 2-bank tiles matching the chunk-interleaved DRAM
                # layout; G streams feed stt consumers directly.
                for c in (0, cw):
                    s = slice(c, c + cw)
                    Prr = ppool.tile([128, 2 * cw], F32, tag="rr")
                    Pbb = ppool.tile([128, 2 * cw], F32, tag="bb")
                    Pge = ppool.tile([128, cw], F32, tag="ge")
                    Pgo = ppool.tile([128, cw], F32, tag="go")
                    nc.tensor.matmul(Prr[:, 0:cw], D05[:], Ee[:, s], start=True, stop=True)
                    nc.tensor.matmul(Prr[:, cw : 2 * cw], D025[:], He[:, s], start=True, stop=True)
                    nc.tensor.matmul(Pgo[:], D025[:], Eo[:, s], start=True, stop=True)
                    nc.tensor.matmul(Pge[:], U025[:], Oe[:, s], start=True, stop=True)
                    nc.tensor.matmul(Pbb[:, 0:cw], U025[:], Ho[:, s], start=True, stop=True)
                    nc.tensor.matmul(Pbb[:, cw : 2 * cw], U05[:], Oo[:, s], start=True, stop=True)

                    # single-op drains into chunk-interleaved R/B blocks
                    if vdrain:
                        V.tensor_copy(RGB[:, R0 + 2 * h + 2 * c : R0 + 2 * h + 2 * c + 2 * cw], Prr[:])
                        V.tensor_copy(RGB[:, B0 + 2 * c : B0 + 2 * c + 2 * cw], Pbb[:])
                    else:
                        S.copy(RGB[:, R0 + 2 * h + 2 * c : R0 + 2 * h + 2 * c + 2 * cw], Prr[:])
                        S.copy(RGB[:, B0 + 2 * c : B0 + 2 * c + 2 * cw], Pbb[:])
                    # G-ee / G-oo
                    V.scalar_tensor_tensor(
                        RGB[:, G0 + c : G0 + c + cw], Hu[:, s], 0.25, Pge[:], mu, ad
                    )
                    V.scalar_tensor_tensor(
                        RGB[:, G0 + 3 * h + c : G0 + 3 * h + c + cw],
                        Hd[:, s], 0.25, Pgo[:], mu, ad,
                    )

                step = 12 * h // store_pieces
                for j in range(store_pieces):
                    eng = S if (alt_stores and j % 2 == 1) else st
                    eng.dma_start(
                        out[q + lo : q + hi, j * step : (j + 1) * step],
                        RGB[lo:hi, j * step : (j + 1) * step],
                    )

    split_sync_waits(nc)
    return nc


_CACHE = {}

TRACE = False
LAST_RESULT = None


def _get_program():
    import os

    key = (
        int(os.environ.get("K6SPLITN", "6")),
        int(os.environ.get("K6INBUFS", "3")),
        int(os.environ.get("K6OUTBUFS", "2")),
        os.environ.get("K6VDRAIN", "0") == "1",
        os.environ.get("K6STALT", "0") == "1",
        os.environ.get("K7LOAD", "swdge"),
    )
    if key not in _CACHE:
        _CACHE[key] = build_program(*key)
    return _CACHE[key]


def kernel(x: np.ndarray) -> np.ndarray:
    global LAST_RESULT
    from concourse.bass_utils import run_bass_kernel_spmd

    n, _, hh, ww = x.shape
    assert (n, hh, ww) == (N_CORES, H, W), x.shape
    nc = _get_program()
    cmm = const_arrays()
    import os
    fp16_in = os.environ.get("K7LOAD", "swdge") == "fp16"
    in_maps = []
    for i in range(N_CORES):
        img = np.rint(x[i, 0] * 255.0).astype(np.uint8)
        # de-interleave: [1024, 2, 1024, 2] -> [p, r, s, i] -> [Ee|Eo|Oe|Oo]
        seg = np.ascontiguousarray(
            img.reshape(NP, 2, HW, 2).transpose(0, 1, 3, 2)
        ).reshape(NP, 4 * HW)
        d = np.zeros((NP, IW), np.float16 if fp16_in else np.uint8)
        d[:, EE0 : EE0 + HW] = seg[:, 0:HW]
        d[:, EO0 : EO0 + HW] = seg[:, HW : 2 * HW]
        d[:, OE0 : OE0 + HW] = seg[:, 2 * HW : 3 * HW]
        d[:, OO0 : OO0 + HW] = seg[:, 3 * HW : 4 * HW]
        in_maps.append({"x": d, "cmm": cmm})
    res = run_bass_kernel_spmd(
        nc, in_maps, core_ids=list(range(N_CORES)), trace=TRACE
    )
    LAST_RESULT = res
    outs = []
    inv = 1.0 / 255.0
    for i in range(N_CORES):
        o = res.results[i]["out"].reshape(NP, 3, 4 * HW)
        # undo chunk interleaving of R (oe/oo) and B (ee/eo) blocks
        ph = np.empty((3, NP, 4, HW), np.uint8)  # [c, p, phase(ee,eo,oe,oo), i]
        R, G, B = o[:, 0], o[:, 1], o[:, 2]
        ph[0, :, 0] = R[:, 0:HW]
        ph[0, :, 1] = R[:, HW : 2 * HW]
        ph[0, :, 2, 0:512] = R[:, 2 * HW : 2 * HW + 512]
        ph[0, :, 3, 0:512] = R[:, 2 * HW + 512 : 3 * HW]
        ph[0, :, 2, 512:] = R[:, 3 * HW : 3 * HW + 512]
        ph[0, :, 3, 512:] = R[:, 3 * HW + 512 : 4 * HW]
        ph[1] = G.reshape(NP, 4, HW)
        ph[2, :, 0, 0:512] = B[:, 0:512]
        ph[2, :, 1, 0:512] = B[:, 512:HW]
        ph[2, :, 0, 512:] = B[:, HW : HW + 512]
        ph[2, :, 1, 512:] = B[:, HW + 512 : 2 * HW]
        ph[2, :, 2] = B[:, 2 * HW : 3 * HW]
        ph[2, :, 3] = B[:, 3 * HW : 4 * HW]
        phr = ph.reshape(3, NP, 2, 2, HW)  # [c, p, r, s, i]
        full = np.ascontiguousarray(
            (phr.astype(np.float32) * inv).transpose(0, 1, 2, 4, 3)
        ).reshape(3, H, W)
        outs.append(full[None])
    return np.concatenate(outs, axis=0)
